# revision 29
# baseline (speedup 1.0000x reference)
"""Trainium2 Bass kernel: CenterHeadIoU 1x1-conv stack.

Computes, for x = ct_feat [B=32, C=128, N=8192]:
  y = relu(bn(sh_w @ x))                       [B, 64, N]
  z_h = relu(bn_h(head_w1[h] @ y)), h=0..5     [B, 64, N] each
  out = concat_h(head_final_w[h] @ z_h + b_h)  [B, 12, N]

Sharding: data-parallel over batch, 4 batches per core on 8 cores;
weights are tiny and replicated. BN is folded into conv weights/biases
on the host; all matmul operands are bf16 (PSUM accumulation stays
fp32, and the rel-err budget of 2e-2 leaves bf16 plenty of margin).

Per 512-column tile the PE runs exactly 7 x 512-row matmuls:
  mm1: w1 [128,64] @ x tile            -> py   [64,512]
  mm2: 3x w2 pair blocks (K=65, the 65th row carries the folded bias
       against a constant-1.0 row of y)  -> pza [128,1024] + pzb [128,512]
  mm3: 3x accumulating pair blocks with slot-expanded lhsT [128,96]
       (the live 12 rows sit at partition offset 12*(t%8), rest zeros)
       so EIGHT tiles' outputs land stacked in ONE psum bank [96,512].
Epilogue is then one ACT Identity(+bias) op per 8 tiles and one striped
DMA straight to DRAM (partition 12s+c -> out[b, c, (j0+s)*512 + f]).

The loop is software-pipelined: iteration t issues mm1 of t+2, mm2 of
t, and mm3 of t-2, so every PE matmul's dependencies resolve at least
a full tile before the PE sequencer decodes it (a late semaphore
blocks the LDWEIGHTS pull-ahead and costs ~100ns of pipeline fill).
PSUM banks: py 2 + pza 2x2 + pzb 1 + po 1 = 8 exactly.

Elementwise work is split so both PSUM-capable engines stay under the
PE's per-tile time: ACT does act1 + the [128,512] z chunk + the
per-8-tile epi; DVE does the [128,1024] z chunk.

The PE starts each execution clock-gated at 1.2 GHz (HAM activity
monitor) and reaches 2.4 GHz only after ~3.4us of sustained activity;
any early idle gap re-throttles it and the re-warm takes a variable
7-60us. The prologue therefore front-loads latency-ordered DMAs
(first x half-tile before everything), pre-warms the PE with dummy
matmuls on a memset scratch tile, and bridges the pipeline-fill gaps
with wait-free dummies. w3s (~590KB) is DMA'd in per-slot chunks so
the first mm3 only waits on the first chunk.

A post-pass moves multi-wait sync conditions onto single-wait NoOp
carriers (this walrus build caps sync waits per instruction).
"""

import os
import sys
import numpy as np

B, C_IN, N, HC = 32, 128, 8192, 64
NCORES = 8
BC = B // NCORES            # batches per core
F = 512                     # free-dim tile = one fp32 PSUM bank
NT = N // F                 # tiles per batch
T = BC * NT                 # tiles per core
GS = 8                      # tiles per mm3 psum group (slots)
EPS = 1e-5
HEAD_OUT = [3, 2, 1, 3, 2, 1]        # hm, reg, height, dim, rot, iou
PAIR_OFF = [0, 5, 9]                 # channel offset of pair p in the 12-ch output

_CACHE = {}
LAST_RESULTS = None
LAST_EXEC_NS = None


def _build_program():
    import concourse.bass as bass
    import concourse.mybir as mybir
    import concourse.tile as tile

    f32 = mybir.dt.float32
    bf16 = mybir.dt.bfloat16
    AF = mybir.ActivationFunctionType

    nc = bass.Bass("TRN2", target_bir_lowering=False, debug=False,
                   num_devices=NCORES)

    x = nc.dram_tensor("x", [BC, C_IN, N], bf16, kind="ExternalInput").ap()
    w1 = nc.dram_tensor("w1", [C_IN, HC], bf16, kind="ExternalInput").ap()
    b1 = nc.dram_tensor("b1", [HC, 1], f32, kind="ExternalInput").ap()
    w2 = nc.dram_tensor("w2", [HC + 1, 384], bf16, kind="ExternalInput").ap()
    w3s = nc.dram_tensor("w3s", [128, GS * 3 * (12 * GS)], bf16,
                         kind="ExternalInput").ap()
    b3x4 = nc.dram_tensor("b3x4", [GS * 12, 1], f32, kind="ExternalInput").ap()
    ones = nc.dram_tensor("ones", [1, F], bf16, kind="ExternalInput").ap()
    out = nc.dram_tensor("out", [BC, 12, N], f32, kind="ExternalOutput").ap()

    with tile.TileContext(nc) as tc:
        with (
            tc.tile_pool(name="consts", bufs=1) as cpool,
            tc.tile_pool(name="xin", bufs=4) as xpool,
            tc.tile_pool(name="ysb", bufs=1) as ypool,
            tc.tile_pool(name="zsb", bufs=4) as zpool,
            tc.tile_pool(name="esb", bufs=2) as epool,
            tc.tile_pool(name="ppy", bufs=2, space="PSUM") as pypool,
            tc.tile_pool(name="pza", bufs=2, space="PSUM") as pzapool,
            tc.tile_pool(name="pzb", bufs=1, space="PSUM") as pzbpool,
            tc.tile_pool(name="ppo", bufs=1, space="PSUM") as popool,
        ):
            NP = T // 2                 # x is DMA'd in 2-tile pairs

            x_tiles = {}

            def load_x(k, split=False):
                b, j2 = divmod(k, NT // 2)
                xt = xpool.tile([C_IN, 2 * F], bf16, name="xt", tag="xt")
                if split:
                    # two DMAs so mm1 of the first tile only waits on half
                    nc.sync.dma_start(out=xt[:, 0:F],
                                      in_=x[b, :, j2 * 2 * F:j2 * 2 * F + F])
                    nc.sync.dma_start(out=xt[:, F:2 * F],
                                      in_=x[b, :, j2 * 2 * F + F:(j2 + 1) * 2 * F])
                else:
                    nc.sync.dma_start(out=xt[:],
                                      in_=x[b, :, j2 * 2 * F:(j2 + 1) * 2 * F])
                x_tiles[k] = xt

            # DMA order is latency-ordered: the operands of the first few
            # tiles (x pair 0, w1, b1) go first so mm1(0) can start early;
            # mm3's tables are only needed ~8 tiles in.
            load_x(0, split=True)
            w1_t = cpool.tile([C_IN, HC], bf16, name="w1_t")
            nc.sync.dma_start(out=w1_t[:], in_=w1[:])
            b1_t = cpool.tile([HC, 1], f32, name="b1_t")
            nc.sync.dma_start(out=b1_t[:], in_=b1[:])
            w2_t = cpool.tile([HC + 1, 384], bf16, name="w2_t")
            nc.sync.dma_start(out=w2_t[:], in_=w2[:])

            # y tiles hoisted: row 64 stays constant 1.0 (feeds the bias row
            # of the K=65 mm2), rows 0..63 rewritten by act1 each iteration.
            y_tiles = []
            for i in range(3):
                y65 = ypool.tile([HC + 1, F], bf16, name=f"y65_{i}",
                                 tag=f"y65_{i}")
                nc.sync.dma_start(out=y65[HC:HC + 1, :], in_=ones[:])
                y_tiles.append(y65)

            # w3s is ~590KB; DMA it in per-slot chunks so mm3 of the first
            # tiles only waits on the first ~74KB (subtile deps), not the
            # whole table — a late w3s idles the PE and re-throttles HAM.
            CW = 3 * 12 * GS
            w3s_t = cpool.tile([128, GS * CW], bf16, name="w3s_t")
            nc.sync.dma_start(out=w3s_t[:, 0:CW], in_=w3s[:, 0:CW])
            b3x4_t = cpool.tile([GS * 12, 1], f32, name="b3x4_t")
            nc.sync.dma_start(out=b3x4_t[:], in_=b3x4[:])
            nc.sync.dma_start(out=w3s_t[:, CW:2 * CW], in_=w3s[:, CW:2 * CW])
            for k in range(1, 4):
                load_x(k)
            for s in range(2, GS):
                nc.sync.dma_start(out=w3s_t[:, s * CW:(s + 1) * CW],
                                  in_=w3s[:, s * CW:(s + 1) * CW])

            def mm1_act1(t):
                import concourse.mybir as mybir
                xt = x_tiles[t // 2]
                xs = xt[:, (t % 2) * F:(t % 2 + 1) * F]
                py = pypool.tile([HC, F], f32, name="py", tag="py")
                nc.tensor.matmul(py[:], w1_t[:], xs, start=True, stop=True)
                y65 = y_tiles[t % 3]
                nc.vector.tensor_scalar(y65[0:HC, :], py[:],
                                        b1_t[:, 0:1], 0.0,
                                        mybir.AluOpType.add,
                                        mybir.AluOpType.max)

            z_tiles = {}

            def mm2_relu(t):
                y65 = y_tiles[t % 3]
                pza = pzapool.tile([128, 2 * F], f32, name="pza", tag="pza")
                pzb = pzbpool.tile([128, F], f32, name="pzb", tag="pzb")
                for p in range(2):
                    nc.tensor.matmul(pza[:, F * p:F * (p + 1)],
                                     w2_t[:, 128 * p:128 * (p + 1)],
                                     y65[:, :], start=True, stop=True)
                nc.tensor.matmul(pzb[:, :], w2_t[:, 256:384], y65[:, :],
                                 start=True, stop=True)
                za = zpool.tile([128, 2 * F], bf16, name="za", tag="za")
                zb = zpool.tile([128, F], bf16, name="zb", tag="zb")
                nc.scalar.activation(za[:, :], pza[:, :], AF.Relu)
                nc.vector.tensor_scalar_max(zb[:, :], pzb[:, :], 0.0)
                z_tiles[t] = (za, zb)

            po_tiles = {}

            def mm3(u):
                s = u % GS
                g = u // GS
                if s == 0:
                    po_tiles[g] = popool.tile([GS * 12, F], f32, name="po",
                                              tag="po")
                po = po_tiles[g]
                za, zb = z_tiles[u]
                rhs = [za[:, 0:F], za[:, F:2 * F], zb[:, :]]
                W = 12 * GS
                for p in range(3):
                    c0 = W * (3 * s + p)
                    nc.tensor.matmul(po[0:GS * 12, :],
                                     w3s_t[:, c0:c0 + W], rhs[p],
                                     start=(s == 0 and p == 0),
                                     stop=(s == GS - 1 and p == 2),
                                     skip_group_check=True)
                del z_tiles[u]

            def epi_out(g):
                po = po_tiles.pop(g)
                e = epool.tile([GS * 12, F], f32, name="e", tag="e")
                nc.scalar.activation(e[0:GS * 12, :], po[0:GS * 12, :],
                                     AF.Identity, bias=b3x4_t[:, 0:1],
                                     scale=1.0)
                b, g4 = divmod(g, NT // GS)
                j0 = g4 * GS
                dview = out[b, :, j0 * F:(j0 + GS) * F]
                dview = dview.rearrange("c (s f) -> s c f", s=GS)
                nc.sync.dma_start(out=dview, in_=e[0:GS * 12, :])

            # Pre-warm: dummy matmuls on a memset scratch tile keep the PE
            # busy while the first x tiles are still in flight — the HAM
            # clock gate releases 2.4 GHz only after ~3.4us of sustained
            # activity, and ANY ~2-3us idle re-throttles it (the re-warm
            # latency is then 7-60us, run-to-run variable). So: start the
            # clock early, and bridge every early pipeline-fill gap.
            scr = cpool.tile([C_IN, F], bf16, name="scr")
            nc.vector.memset(scr[:], 0.0)
            for i in range(6):
                pd = pypool.tile([HC, F], f32, name="pd", tag="py")
                nc.tensor.matmul(pd[:], scr[:, 0:HC], scr[:],
                                 start=True, stop=True)

            # Prologue: prime y for tiles 0-1 (x pairs 0-3 already in
            # flight); the loop then runs mm1 two tiles ahead so all its
            # semaphores fire well before the PE sequencer decodes it.
            mm1_act1(0)
            mm1_act1(1)
            # Wait-free bridge dummies (pzb tag: no readers pending, pure
            # WAW) so the PE never idles while act1(0)/the first DVE relu
            # propagate their semaphores.
            for i in range(3):
                pb = pzbpool.tile([128, F], f32, name="pb", tag="pzb")
                nc.tensor.matmul(pb[:], scr[:, 0:128], scr[:],
                                 start=True, stop=True)

            # Two-tile-deep software pipeline: iteration t runs mm1/act1 of
            # t+1, mm2/relu of t, and mm3 of t-2, so every PE matmul's
            # inputs were produced at least one full tile earlier.
            def post_mm3(u):
                mm3(u)
                if u % GS == GS - 1:
                    epi_out(u // GS)

            for t in range(T):
                if t % 2 == 0 and t // 2 + 4 < NP:
                    load_x(t // 2 + 4)
                if t + 2 < T:
                    mm1_act1(t + 2)
                mm2_relu(t)
                if t >= 2:
                    post_mm3(t - 2)
            for u in (T - 2, T - 1):
                post_mm3(u)
    _split_waits(nc)
    return nc


def _split_waits(nc, cap=1):
    """This container's walrus build rejects instructions carrying more than
    a small number of sync waits (matmuls: just one). Move excess waits onto
    single-wait NoOp carriers inserted before the instruction on the same
    engine — semantically identical (conjunction of waits, in-order
    sequencers)."""
    import concourse.mybir as mybir

    k = 0
    for func in nc.m.functions:
        for bb in func.blocks:
            insts = bb.instructions
            out_insts = []
            changed = False
            for inst in insts:
                si = inst.sync_info
                waits = list(si.on_wait) if si and si.on_wait else []
                if len(waits) > cap:
                    for w in waits[:-cap]:
                        d = mybir.InstNoOp(name=f"I-sw{k}", ins=[], outs=[])
                        k += 1
                        d.engine = inst.engine
                        d.sync_info = mybir.SyncInfo(on_wait=[w], on_update=[])
                        nc.register_instruction(d)
                        out_insts.append(d)
                    inst.sync_info = mybir.SyncInfo(
                        on_wait=waits[-cap:],
                        on_update=list(si.on_update) if si.on_update else [])
                    changed = True
                out_insts.append(inst)
            if changed:
                bb.instructions = out_insts


# revision 30
# speedup vs baseline: 1.0849x; 1.0849x over previous
"""Trainium2 Bass kernel: CenterHeadIoU 1x1-conv stack.

Computes, for x = ct_feat [B=32, C=128, N=8192]:
  y = relu(bn(sh_w @ x))                       [B, 64, N]
  z_h = relu(bn_h(head_w1[h] @ y)), h=0..5     [B, 64, N] each
  out = concat_h(head_final_w[h] @ z_h + b_h)  [B, 12, N]

Sharding: data-parallel over batch, 4 batches per core on 8 cores;
weights are tiny and replicated. BN is folded into conv weights/biases
on the host; all matmul operands are bf16 (PSUM accumulation stays
fp32, and the rel-err budget of 2e-2 leaves bf16 plenty of margin).

Per 512-column tile the PE runs exactly 7 x 512-row matmuls:
  mm1: w1 [128,64] @ x tile            -> py   [64,512]
  mm2: 3x w2 pair blocks (K=65, the 65th row carries the folded bias
       against a constant-1.0 row of y)  -> pza [128,1024] + pzb [128,512]
  mm3: 3x accumulating pair blocks with slot-expanded lhsT [128,96]
       (the live 12 rows sit at partition offset 12*(t%8), rest zeros)
       so EIGHT tiles' outputs land stacked in ONE psum bank [96,512].
Epilogue is then one ACT Identity(+bias) op per 8 tiles and one striped
DMA straight to DRAM (partition 12s+c -> out[b, c, (j0+s)*512 + f]).

The loop is software-pipelined: iteration t issues mm1 of t+2, mm2 of
t, and mm3 of t-2, so every PE matmul's dependencies resolve at least
a full tile before the PE sequencer decodes it (a late semaphore
blocks the LDWEIGHTS pull-ahead and costs ~100ns of pipeline fill).
PSUM banks: py 2 + pza 2x2 + pzb 1 + po 1 = 8 exactly.

Elementwise work is split so both PSUM-capable engines stay under the
PE's per-tile time: ACT does act1 + the [128,512] z chunk + the
per-8-tile epi; DVE does the [128,1024] z chunk.

The PE starts each execution clock-gated at 1.2 GHz (HAM activity
monitor) and reaches 2.4 GHz only after ~3.4us of sustained activity;
any early idle gap re-throttles it and the re-warm takes a variable
7-60us. The prologue therefore front-loads latency-ordered DMAs
(first x half-tile before everything), pre-warms the PE with dummy
matmuls on a memset scratch tile, and bridges the pipeline-fill gaps
with wait-free dummies. w3s (~590KB) is DMA'd in per-slot chunks so
the first mm3 only waits on the first chunk.

A post-pass moves multi-wait sync conditions onto single-wait NoOp
carriers (this walrus build caps sync waits per instruction).
"""

import os
import sys
import numpy as np

B, C_IN, N, HC = 32, 128, 8192, 64
NCORES = 8
BC = B // NCORES            # batches per core
F = 512                     # free-dim tile = one fp32 PSUM bank
NT = N // F                 # tiles per batch
T = BC * NT                 # tiles per core
GS = 8                      # tiles per mm3 psum group (slots)
EPS = 1e-5
HEAD_OUT = [3, 2, 1, 3, 2, 1]        # hm, reg, height, dim, rot, iou
PAIR_OFF = [0, 5, 9]                 # channel offset of pair p in the 12-ch output

_CACHE = {}
LAST_RESULTS = None
LAST_EXEC_NS = None


def _build_program():
    import concourse.bass as bass
    import concourse.mybir as mybir
    import concourse.tile as tile

    f32 = mybir.dt.float32
    bf16 = mybir.dt.bfloat16
    AF = mybir.ActivationFunctionType

    nc = bass.Bass("TRN2", target_bir_lowering=False, debug=False,
                   num_devices=NCORES)

    x = nc.dram_tensor("x", [BC, C_IN, N], bf16, kind="ExternalInput").ap()
    w1 = nc.dram_tensor("w1", [C_IN, HC], bf16, kind="ExternalInput").ap()
    b1 = nc.dram_tensor("b1", [HC, 1], f32, kind="ExternalInput").ap()
    w2 = nc.dram_tensor("w2", [HC + 1, 384], bf16, kind="ExternalInput").ap()
    w3s = nc.dram_tensor("w3s", [128, GS * 3 * (12 * GS)], bf16,
                         kind="ExternalInput").ap()
    b3x4 = nc.dram_tensor("b3x4", [GS * 12, 1], f32, kind="ExternalInput").ap()
    ones = nc.dram_tensor("ones", [1, F], bf16, kind="ExternalInput").ap()
    out = nc.dram_tensor("out", [BC, 12, N], f32, kind="ExternalOutput").ap()

    with tile.TileContext(nc) as tc:
        with (
            tc.tile_pool(name="consts", bufs=1) as cpool,
            tc.tile_pool(name="xin", bufs=4) as xpool,
            tc.tile_pool(name="ysb", bufs=1) as ypool,
            tc.tile_pool(name="zsb", bufs=4) as zpool,
            tc.tile_pool(name="esb", bufs=2) as epool,
            tc.tile_pool(name="ppy", bufs=2, space="PSUM") as pypool,
            tc.tile_pool(name="pza", bufs=2, space="PSUM") as pzapool,
            tc.tile_pool(name="pzb", bufs=1, space="PSUM") as pzbpool,
            tc.tile_pool(name="ppo", bufs=1, space="PSUM") as popool,
        ):
            NP = T // 2                 # x is DMA'd in 2-tile pairs

            x_tiles = {}

            def load_x(k, split=False):
                b, j2 = divmod(k, NT // 2)
                xt = xpool.tile([C_IN, 2 * F], bf16, name="xt", tag="xt")
                if split:
                    # two DMAs so mm1 of the first tile only waits on half
                    nc.sync.dma_start(out=xt[:, 0:F],
                                      in_=x[b, :, j2 * 2 * F:j2 * 2 * F + F])
                    nc.sync.dma_start(out=xt[:, F:2 * F],
                                      in_=x[b, :, j2 * 2 * F + F:(j2 + 1) * 2 * F])
                else:
                    nc.sync.dma_start(out=xt[:],
                                      in_=x[b, :, j2 * 2 * F:(j2 + 1) * 2 * F])
                x_tiles[k] = xt

            # DMA order is latency-ordered: the operands of the first few
            # tiles (x pair 0, w1, b1) go first so mm1(0) can start early;
            # mm3's tables are only needed ~8 tiles in.
            load_x(0, split=True)
            w1_t = cpool.tile([C_IN, HC], bf16, name="w1_t")
            nc.sync.dma_start(out=w1_t[:], in_=w1[:])
            b1_t = cpool.tile([HC, 1], f32, name="b1_t")
            nc.sync.dma_start(out=b1_t[:], in_=b1[:])
            w2_t = cpool.tile([HC + 1, 384], bf16, name="w2_t")
            nc.sync.dma_start(out=w2_t[:], in_=w2[:])

            # y tiles hoisted: row 64 stays constant 1.0 (feeds the bias row
            # of the K=65 mm2), rows 0..63 rewritten by act1 each iteration.
            y_tiles = []
            for i in range(3):
                y65 = ypool.tile([HC + 1, F], bf16, name=f"y65_{i}",
                                 tag=f"y65_{i}")
                nc.sync.dma_start(out=y65[HC:HC + 1, :], in_=ones[:])
                y_tiles.append(y65)

            # w3s is ~590KB; DMA it in per-slot chunks so mm3 of the first
            # tiles only waits on the first ~74KB (subtile deps), not the
            # whole table — a late w3s idles the PE and re-throttles HAM.
            CW = 3 * 12 * GS
            w3s_t = cpool.tile([128, GS * CW], bf16, name="w3s_t")
            nc.sync.dma_start(out=w3s_t[:, 0:CW], in_=w3s[:, 0:CW])
            b3x4_t = cpool.tile([GS * 12, 1], f32, name="b3x4_t")
            nc.sync.dma_start(out=b3x4_t[:], in_=b3x4[:])
            nc.sync.dma_start(out=w3s_t[:, CW:2 * CW], in_=w3s[:, CW:2 * CW])
            for k in range(1, 4):
                load_x(k)
            for s in range(2, GS):
                nc.sync.dma_start(out=w3s_t[:, s * CW:(s + 1) * CW],
                                  in_=w3s[:, s * CW:(s + 1) * CW])

            def mm1_act1(t):
                xt = x_tiles[t // 2]
                xs = xt[:, (t % 2) * F:(t % 2 + 1) * F]
                py = pypool.tile([HC, F], f32, name="py", tag="py")
                nc.tensor.matmul(py[:], w1_t[:], xs, start=True, stop=True)
                y65 = y_tiles[t % 3]
                nc.scalar.activation(y65[0:HC, :], py[:], AF.Relu,
                                     bias=b1_t[:, 0:1], scale=1.0)

            z_tiles = {}

            def mm2_relu(t):
                y65 = y_tiles[t % 3]
                pza = pzapool.tile([128, 2 * F], f32, name="pza", tag="pza")
                pzb = pzbpool.tile([128, F], f32, name="pzb", tag="pzb")
                for p in range(2):
                    nc.tensor.matmul(pza[:, F * p:F * (p + 1)],
                                     w2_t[:, 128 * p:128 * (p + 1)],
                                     y65[:, :], start=True, stop=True)
                nc.tensor.matmul(pzb[:, :], w2_t[:, 256:384], y65[:, :],
                                 start=True, stop=True)
                za = zpool.tile([128, 2 * F], bf16, name="za", tag="za")
                zb = zpool.tile([128, F], bf16, name="zb", tag="zb")
                nc.vector.tensor_scalar_max(za[:, :], pza[:, :], 0.0)
                nc.scalar.activation(zb[:, :], pzb[:, :], AF.Relu)
                z_tiles[t] = (za, zb)

            po_tiles = {}

            def mm3(u):
                s = u % GS
                g = u // GS
                if s == 0:
                    po_tiles[g] = popool.tile([GS * 12, F], f32, name="po",
                                              tag="po")
                po = po_tiles[g]
                za, zb = z_tiles[u]
                rhs = [za[:, 0:F], za[:, F:2 * F], zb[:, :]]
                W = 12 * GS
                for p in range(3):
                    c0 = W * (3 * s + p)
                    nc.tensor.matmul(po[0:GS * 12, :],
                                     w3s_t[:, c0:c0 + W], rhs[p],
                                     start=(s == 0 and p == 0),
                                     stop=(s == GS - 1 and p == 2),
                                     skip_group_check=True)
                del z_tiles[u]

            def epi_out(g):
                po = po_tiles.pop(g)
                e = epool.tile([GS * 12, F], f32, name="e", tag="e")
                nc.scalar.activation(e[0:GS * 12, :], po[0:GS * 12, :],
                                     AF.Identity, bias=b3x4_t[:, 0:1],
                                     scale=1.0)
                b, g4 = divmod(g, NT // GS)
                j0 = g4 * GS
                dview = out[b, :, j0 * F:(j0 + GS) * F]
                dview = dview.rearrange("c (s f) -> s c f", s=GS)
                nc.sync.dma_start(out=dview, in_=e[0:GS * 12, :])

            # Pre-warm: dummy matmuls on a memset scratch tile keep the PE
            # busy while the first x tiles are still in flight — the HAM
            # clock gate releases 2.4 GHz only after ~3.4us of sustained
            # activity, and ANY ~2-3us idle re-throttles it (the re-warm
            # latency is then 7-60us, run-to-run variable). So: start the
            # clock early, and bridge every early pipeline-fill gap.
            scr = cpool.tile([C_IN, F], bf16, name="scr")
            nc.vector.memset(scr[:], 0.0)
            for i in range(6):
                pd = pypool.tile([HC, F], f32, name="pd", tag="py")
                nc.tensor.matmul(pd[:], scr[:, 0:HC], scr[:],
                                 start=True, stop=True)

            # Prologue: prime y for tiles 0-1 (x pairs 0-3 already in
            # flight); the loop then runs mm1 two tiles ahead so all its
            # semaphores fire well before the PE sequencer decodes it.
            mm1_act1(0)
            mm1_act1(1)
            # Wait-free bridge dummies (pzb tag: no readers pending, pure
            # WAW) so the PE never idles while act1(0)/the first DVE relu
            # propagate their semaphores.
            for i in range(3):
                pb = pzbpool.tile([128, F], f32, name="pb", tag="pzb")
                nc.tensor.matmul(pb[:], scr[:, 0:128], scr[:],
                                 start=True, stop=True)

            # Two-tile-deep software pipeline: iteration t runs mm1/act1 of
            # t+1, mm2/relu of t, and mm3 of t-2, so every PE matmul's
            # inputs were produced at least one full tile earlier.
            def post_mm3(u):
                mm3(u)
                if u % GS == GS - 1:
                    epi_out(u // GS)

            for t in range(T):
                if t % 2 == 0 and t // 2 + 4 < NP:
                    load_x(t // 2 + 4)
                if t + 2 < T:
                    mm1_act1(t + 2)
                mm2_relu(t)
                if t >= 2:
                    post_mm3(t - 2)
            for u in (T - 2, T - 1):
                post_mm3(u)
    _split_waits(nc)
    return nc


def _split_waits(nc, cap=1):
    """This container's walrus build rejects instructions carrying more than
    a small number of sync waits (matmuls: just one). Move excess waits onto
    single-wait NoOp carriers inserted before the instruction on the same
    engine — semantically identical (conjunction of waits, in-order
    sequencers)."""
    import concourse.mybir as mybir

    k = 0
    for func in nc.m.functions:
        for bb in func.blocks:
            insts = bb.instructions
            out_insts = []
            changed = False
            for inst in insts:
                si = inst.sync_info
                waits = list(si.on_wait) if si and si.on_wait else []
                if len(waits) > cap:
                    for w in waits[:-cap]:
                        d = mybir.InstNoOp(name=f"I-sw{k}", ins=[], outs=[])
                        k += 1
                        d.engine = inst.engine
                        d.sync_info = mybir.SyncInfo(on_wait=[w], on_update=[])
                        nc.register_instruction(d)
                        out_insts.append(d)
                    inst.sync_info = mybir.SyncInfo(
                        on_wait=waits[-cap:],
                        on_update=list(si.on_update) if si.on_update else [])
                    changed = True
                out_insts.append(inst)
            if changed:
                bb.instructions = out_insts
